# revision 26
# baseline (speedup 1.0000x reference)
"""AFSDRefineHead distributed Trainium2 kernel (8 NeuronCores).

Sharding: cores 0-3 = loc branch, 4-7 = conf branch. Within a branch group,
core s owns a 128-channel output shard of every conv. The prop conv (2048->512)
is input-sharded: partial sums are AllReduce'd within each branch group.
frame_level_feat boundary pooling is sharded (batch, channel-quarter) across
all 8 cores and exchanged with one 8-way AllGather.

Segment max pooling = bf16 sparse tables (doubling windows) staged in DRAM +
indirect-DMA gathers with host-precomputed (level, offset) row indices.
"""

import sys
import numpy as np

for _p in ("/opt/trn_rl_repo",):
    if _p not in sys.path:
        sys.path.append(_p)

P = 128
B = 2
N = 128          # proposals (= T_lvl)
TF = 512         # frame-level T
CIN = 512
NLF = 5          # stored (even) levels for TF=512 tables: k in {0,2,4,6,8}
NLL = 4          # stored (even) levels for N=128 tables: k in {0,2,4,6}
EPS = 1e-5

_NC_CACHE = {}

TRACE = False
LAST_EXEC_NS = None
LAST_RESULTS = None


def _build_nc():
    import concourse.bass as bass
    import concourse.bacc as bacc
    import concourse.tile as tile
    from concourse import mybir
    from concourse.masks import make_identity

    FP = mybir.dt.float32
    Bb = mybir.dt.bfloat16
    I32 = mybir.dt.int32
    AX = mybir.AxisListType
    ALU = mybir.AluOpType
    AF = mybir.ActivationFunctionType

    nc = bacc.Bacc(None, debug=False, target_bir_lowering=False)

    # ---------------- I/O ----------------
    feat = nc.dram_tensor("feat", [CIN, 2 * N], Bb, kind="ExternalInput")
    flfs = nc.dram_tensor("flfs", [P, TF], FP, kind="ExternalInput")
    wcur = nc.dram_tensor("wcur", [CIN, P], Bb, kind="ExternalInput")
    wlr = nc.dram_tensor("wlr", [CIN, 2 * P], Bb, kind="ExternalInput")
    wroi = nc.dram_tensor("wroi", [CIN, P], Bb, kind="ExternalInput")
    wprop = nc.dram_tensor("wprop", [CIN, 512], Bb, kind="ExternalInput")
    whead = nc.dram_tensor("whead", [CIN, 24], Bb, kind="ExternalInput")
    wcen = nc.dram_tensor("wcen", [CIN, 3], Bb, kind="ExternalInput")
    gng = nc.dram_tensor("gng", [P, 9], FP, kind="ExternalInput")
    gnb = nc.dram_tensor("gnb", [P, 9], FP, kind="ExternalInput")
    gidx = nc.dram_tensor("gidx", [P, 20], I32, kind="ExternalInput")

    oA = nc.dram_tensor("oA", [TF, P], FP, kind="ExternalOutput")
    oB = nc.dram_tensor("oB", [B, N, P], FP, kind="ExternalOutput")
    oC = nc.dram_tensor("oC", [B, N, P], FP, kind="ExternalOutput")
    oH = nc.dram_tensor("oH", [35, 2 * N], FP, kind="ExternalOutput")

    # ---------------- internal DRAM ----------------
    tblF = nc.dram_tensor("tblF", [NLF * TF, P], Bb)
    tblL = nc.dram_tensor("tblL", [4 * NLL * N, P], Bb)
    agin = nc.dram_tensor("agin", [P, P], Bb)
    agout = nc.dram_tensor("agout", [8 * P, P], Bb, addr_space="Shared")
    arin = nc.dram_tensor("arin", [512, 2 * N], Bb)
    arout = nc.dram_tensor("arout", [512, 2 * N], Bb)

    from contextlib import ExitStack
    with tile.TileContext(nc) as tc, ExitStack() as ctx:
        sb = ctx.enter_context(tc.tile_pool(name="sb", bufs=1))
        sb2 = ctx.enter_context(tc.tile_pool(name="sb2", bufs=2))
        pmm = ctx.enter_context(tc.tile_pool(name="pmm", bufs=2, space="PSUM"))
        ptr = ctx.enter_context(tc.tile_pool(name="ptr", bufs=2, space="PSUM"))
        pt = ctx.enter_context(tc.tile_pool(name="pt", bufs=2, space="PSUM"))

        # ---------- constants (NEFF-embedded) ----------
        from ml_dtypes import bfloat16 as np_bf16

        def const_tile(arr, dt, name):
            d = nc.inline_tensor(np.ascontiguousarray(arr), name=name)
            t = sb.tile(list(arr.shape), dt, tag=name)
            nc.sync.dma_start(out=t[:], in_=d[:, :])
            return t

        eyeF = const_tile(np.eye(P, dtype=np.float32), FP, "eyeF")
        eyeB = const_tile(np.eye(P, dtype=np.float32).astype(np_bf16), Bb, "eyeB")

        def expmat(Gn, gs, transposed):
            m = np.zeros((P, Gn), np.float32)
            for c in range(P):
                m[c, c // gs] = 1.0
            return m.T.copy() if transposed else m

        E16 = const_tile(expmat(8, 16, False), FP, "E16")
        ET16 = const_tile(expmat(8, 16, True), FP, "ET16")
        E32 = const_tile(expmat(4, 32, False), FP, "E32")
        ET32 = const_tile(expmat(4, 32, True), FP, "ET32")
        epsT = const_tile(np.full((P, 1), EPS, np.float32), FP, "epsT")

        # ---------- input loads ----------
        def load(dram, shape, dt, name, ktiles=None):
            t = sb.tile(shape, dt, tag=name)
            src = dram[:, :]
            if ktiles is not None:
                src = src.rearrange("(k p) n -> p k n", p=P)
            nc.sync.dma_start(out=t[:], in_=src)
            return t

        feat_s = load(feat, [P, 4, 2 * N], Bb, "feat_s", ktiles=4)
        flfs_s = load(flfs, [P, TF], FP, "flfs_s")
        wcur_s = load(wcur, [P, 4, P], Bb, "wcur_s", ktiles=4)
        wlr_s = load(wlr, [P, 4, 2 * P], Bb, "wlr_s", ktiles=4)
        wroi_s = load(wroi, [P, 4, P], Bb, "wroi_s", ktiles=4)
        wprop_s = load(wprop, [P, 4, 512], Bb, "wprop_s", ktiles=4)
        whead_s = load(whead, [P, 4, 24], Bb, "whead_s", ktiles=4)
        wcen_s = load(wcen, [P, 4, 3], Bb, "wcen_s", ktiles=4)
        gng_s = load(gng, [P, 9], FP, "gng_s")
        gnb_s = load(gnb, [P, 9], FP, "gnb_s")
        gidx_s = load(gidx, [P, 20], I32, "gidx_s")

        # ---------- flf transposes -> oA ----------
        oA_sb = sb.tile([P, 4, P], FP, tag="oA_sb")
        for i in range(4):
            tp = ptr.tile([P, P], FP, tag="tr")
            nc.tensor.transpose(tp[:], flfs_s[:, i * P:(i + 1) * P], eyeF[:])
            nc.scalar.copy(oA_sb[:, i, :], tp[:])
        nc.sync.dma_start(out=oA[:, :].rearrange("(i p) c -> p i c", p=P), in_=oA_sb[:])

        # ---------- flf sparse table ----------
        # All levels built channel-major (free-dim shifts only); even levels
        # PE-transposed into t-major FTf and written to DRAM for the gather.
        FTf = sb.tile([P, NLF, 4, P], Bb, tag="FTf")
        Lch = sb.tile([P, 9, TF], Bb, tag="Lch")
        nc.vector.tensor_copy(Lch[:, 0, :], flfs_s[:])
        for k in range(1, 9):
            h = 1 << (k - 1)
            nc.vector.tensor_max(Lch[:, k, 0:TF - h], Lch[:, k - 1, 0:TF - h],
                                 Lch[:, k - 1, h:TF])
            nc.vector.tensor_copy(Lch[:, k, TF - h:TF], Lch[:, k - 1, TF - h:TF])
        for li in range(NLF):
            tpb = ptr.tile([P, 4, P], Bb, tag="trb")
            for i in range(4):
                nc.tensor.transpose(tpb[:, i, :], Lch[:, 2 * li, i * P:(i + 1) * P],
                                    eyeB[:])
            if li % 2 == 0:
                nc.scalar.copy(FTf[:, li, :, :], tpb[:])
            else:
                nc.vector.tensor_copy(FTf[:, li, :, :], tpb[:])
        nc.sync.dma_start(
            out=tblF[:, :].rearrange("(k i p) c -> p k i c", p=P, i=4),
            in_=FTf[:])

        # ---------- flf gather + pool + AllGather ----------
        def gather(tbl, col, tagn):
            g = sb2.tile([P, P], Bb, tag=tagn)
            nc.gpsimd.indirect_dma_start(
                out=g[:], out_offset=None, in_=tbl[:, :],
                in_offset=bass.IndirectOffsetOnAxis(ap=gidx_s[:, col:col + 1], axis=0))
            return g

        gf = [gather(tblF, j, "gf%d" % j) for j in range(4)]
        pF = sb.tile([P, P], Bb, tag="pF")
        pFx = sb.tile([P, P], Bb, tag="pFx")
        nc.vector.tensor_max(pFx[:], gf[0][:], gf[1][:])
        nc.vector.tensor_max(pFx[:], pFx[:], gf[2][:])
        nc.vector.tensor_max(pF[:], pFx[:], gf[3][:])
        tpF = ptr.tile([P, P], Bb, tag="tr")
        nc.tensor.transpose(tpF[:], pF[:], eyeB[:])
        pFT = sb.tile([P, P], Bb, tag="pFT")
        nc.scalar.copy(pFT[:], tpF[:])
        nc.sync.dma_start(out=agin[:, :], in_=pFT[:])
        nc.gpsimd.collective_compute(
            "AllGather", ALU.bypass, replica_groups=[list(range(8))],
            ins=[agin[:, :]], outs=[agout[:, :]])
        rhsF = sb.tile([P, 4, 2 * N], Bb, tag="rhsF")
        for q in range(4):
            for b in range(B):
                blk = b * 4 + q
                nc.sync.dma_start(out=rhsF[:, q, b * N:(b + 1) * N],
                                  in_=agout[blk * P:(blk + 1) * P, :])

        # ---------- GroupNorm helper ----------
        def gn_relu(src_ap, gcol, G, out_ap):
            # src_ap: [128, 256] f32 (psum) or bf16 (sbuf); GN per (group, batch)
            # over (128/G chans x 128 cols); ReLU; write to out_ap.
            gsz = P // G
            E, ET = (E16, ET16) if gsz == 16 else (E32, ET32)
            cnt = float(gsz * N)
            y = sb2.tile([P, 2 * N], FP, tag="gn_y")
            nc.scalar.copy(y[:], src_ap)
            ysq = sb2.tile([P, 2 * N], FP, tag="gn_ysq")
            nc.vector.tensor_tensor(ysq[:], y[:], y[:], op=ALU.mult)
            st = sb2.tile([P, 4], FP, tag="gn_st")
            nc.vector.reduce_sum(st[:, 0:2], y[:].rearrange("p (b n) -> p b n", b=2), axis=AX.X)
            nc.vector.reduce_sum(st[:, 2:4], ysq[:].rearrange("p (b n) -> p b n", b=2), axis=AX.X)
            gstat = pt.tile([G, 4], FP, tag="tiny")
            nc.tensor.matmul(gstat[:], lhsT=E[:, :], rhs=st[:], start=True, stop=True)
            gs = sb2.tile([G, 8], FP, tag="gn_gs")
            nc.scalar.mul(gs[:, 0:4], gstat[:], 1.0 / cnt)      # mu(2) | msq(2)
            nc.vector.tensor_tensor(gs[:, 4:6], gs[:, 0:2], gs[:, 0:2], op=ALU.mult)
            nc.vector.tensor_sub(gs[:, 4:6], gs[:, 2:4], gs[:, 4:6])   # var
            nc.scalar.activation(gs[:, 6:8], gs[:, 4:6], AF.Sqrt, bias=epsT[0:G, :])
            nc.vector.reciprocal(gs[:, 4:6], gs[:, 6:8])        # rstd
            nc.scalar.mul(gs[:, 6:8], gs[:, 0:2], -1.0)         # -mu
            ex = pt.tile([P, 4], FP, tag="tiny")
            nc.tensor.matmul(ex[:], lhsT=ET[:, :], rhs=gs[:, 4:8], start=True, stop=True)
            sc = sb2.tile([P, 4], FP, tag="gn_sc")
            nc.vector.tensor_scalar_mul(sc[:, 0:2], ex[:, 0:2], gng_s[:, gcol:gcol + 1])
            nc.vector.tensor_tensor(sc[:, 2:4], ex[:, 2:4], sc[:, 0:2], op=ALU.mult)
            nc.vector.tensor_scalar_add(sc[:, 2:4], sc[:, 2:4], gnb_s[:, gcol:gcol + 1])
            for b in range(B):
                nc.scalar.activation(out_ap[:, b * N:(b + 1) * N],
                                     y[:, b * N:(b + 1) * N], AF.Relu,
                                     bias=sc[:, 2 + b:3 + b], scale=sc[:, b:b + 1])

        propRhs = sb.tile([P, 4, 2 * N], Bb, tag="propRhs")
        propGN = sb.tile([P, 4, 2 * N], Bb, tag="propGN")

        # ---------- cur conv ----------
        ps = pmm.tile([P, 2 * N], FP, tag="mm")
        for k in range(4):
            nc.tensor.matmul(ps[:], lhsT=wcur_s[:, k, :], rhs=feat_s[:, k, :],
                             start=(k == 0), stop=(k == 3))
        gn_relu(ps[:], 0, 8, propRhs[:, 3, :])

        # ---------- lr conv (2 M-tiles) + feat2 outputs + lr tables ----------
        FTl = sb.tile([P, 4, NLL, P], Bb, tag="FTl")
        lrf = sb.tile([P, 2, 2 * N], FP, tag="lrf")
        for m in range(2):
            ps = pmm.tile([P, 2 * N], FP, tag="mm")
            for k in range(4):
                nc.tensor.matmul(ps[:], lhsT=wlr_s[:, k, m * P:(m + 1) * P],
                                 rhs=feat_s[:, k, :], start=(k == 0), stop=(k == 3))
            gn_relu(ps[:], 1 + m, 4, lrf[:, m, :])
        oBC = [oB, oC]
        for m in range(2):
            for b in range(B):
                tp = ptr.tile([P, P], FP, tag="tr")
                nc.tensor.transpose(tp[:], lrf[:, m, b * N:(b + 1) * N], eyeF[:])
                ob_sb = sb2.tile([P, P], FP, tag="obc")
                nc.scalar.copy(ob_sb[:], tp[:])
                nc.sync.dma_start(out=oBC[m][b, :, :], in_=ob_sb[:])
        # lr tables: all levels channel-major, even levels transposed to FTl.
        # sub-table index = b*2 + half, matching host gidx rows.
        LchL = sb.tile([P, 4, 7, N], Bb, tag="LchL")  # [ch, sub=(b,half), lvl, n]
        for b in range(B):
            for m in range(2):
                nc.vector.tensor_copy(LchL[:, b * 2 + m, 0, :],
                                      lrf[:, m, b * N:(b + 1) * N])
        for k in range(1, 7):
            h = 1 << (k - 1)
            nc.vector.tensor_max(LchL[:, :, k, 0:N - h],
                                 LchL[:, :, k - 1, 0:N - h],
                                 LchL[:, :, k - 1, h:N])
            nc.vector.tensor_copy(LchL[:, :, k, N - h:N],
                                  LchL[:, :, k - 1, N - h:N])
        for li in range(NLL):
            tpb = ptr.tile([P, 4, P], Bb, tag="trb")
            for s2 in range(4):
                nc.tensor.transpose(tpb[:, s2, :], LchL[:, s2, 2 * li, :], eyeB[:])
            if li % 2 == 0:
                nc.scalar.copy(FTl[:, :, li, :], tpb[:])
            else:
                nc.vector.tensor_copy(FTl[:, :, li, :], tpb[:])
        nc.sync.dma_start(
            out=tblL[:, :].rearrange("(s k p) c -> p s k c", p=P, k=NLL),
            in_=FTl[:])

        # ---------- lr gathers -> pooled -> prop rhs tiles 1,2 ----------
        for b in range(B):
            for half in range(2):
                sub = b * 2 + half
                gl = [gather(tblL, 4 + 4 * sub + j, "gl%d" % j) for j in range(4)]
                pl = sb2.tile([P, P], Bb, tag="pl")
                plx = sb2.tile([P, P], Bb, tag="plx")
                nc.vector.tensor_max(plx[:], gl[0][:], gl[1][:])
                nc.vector.tensor_max(plx[:], plx[:], gl[2][:])
                nc.vector.tensor_max(pl[:], plx[:], gl[3][:])
                tp = ptr.tile([P, P], Bb, tag="tr")
                nc.tensor.transpose(tp[:], pl[:], eyeB[:])
                nc.scalar.copy(propRhs[:, 1 + half, b * N:(b + 1) * N], tp[:])

        # ---------- roi conv ----------
        ps = pmm.tile([P, 2 * N], FP, tag="mm")
        for k in range(4):
            nc.tensor.matmul(ps[:], lhsT=wroi_s[:, k, :], rhs=rhsF[:, k, :],
                             start=(k == 0), stop=(k == 3))
        gn_relu(ps[:], 3, 8, propRhs[:, 0, :])

        # ---------- prop conv partials + AllReduce ----------
        for m in range(4):
            ps = pmm.tile([P, 2 * N], FP, tag="mm")
            for k in range(4):
                nc.tensor.matmul(ps[:], lhsT=wprop_s[:, k, m * P:(m + 1) * P],
                                 rhs=propRhs[:, k, :], start=(k == 0), stop=(k == 3))
            pp = sb2.tile([P, 2 * N], Bb, tag="pp")
            nc.scalar.copy(pp[:], ps[:])
            nc.sync.dma_start(out=arin[m * P:(m + 1) * P, :], in_=pp[:])
        nc.gpsimd.collective_compute(
            "AllReduce", ALU.add, replica_groups=[[0, 1, 2, 3], [4, 5, 6, 7]],
            ins=[arin[:, :]], outs=[arout[:, :]])
        prop_sb = sb.tile([P, 4, 2 * N], Bb, tag="prop_sb")
        for m in range(4):
            nc.sync.dma_start(out=prop_sb[:, m, :], in_=arout[m * P:(m + 1) * P, :])
            gn_relu(prop_sb[:, m, :], 4 + m, 8, propGN[:, m, :])

        # ---------- heads ----------
        hps = pmm.tile([24, 2 * N], FP, tag="mm")
        cps = pt.tile([3, 2 * N], FP, tag="tiny")
        for k in range(4):
            nc.tensor.matmul(hps[:], lhsT=whead_s[:, k, :], rhs=propGN[:, k, :],
                             start=(k == 0), stop=(k == 3))
            nc.tensor.matmul(cps[:], lhsT=wcen_s[:, k, :], rhs=propGN[:, k, :],
                             start=(k == 0), stop=(k == 3))
        oH_sb = sb.tile([35, 2 * N], FP, tag="oH_sb")
        nc.scalar.copy(oH_sb[0:24, :], hps[:])
        nc.scalar.copy(oH_sb[32:35, :], cps[:])
        nc.sync.dma_start(out=oH[:, :], in_=oH_sb[:])

    nc.compile()
    return nc


def _get_nc():
    if "nc" not in _NC_CACHE:
        _NC_CACHE["nc"] = _build_nc()
    return _NC_CACHE["nc"]


def _install_ntff_hook():
    """Register the axon NTFF profile hook (the agent image lacks
    antenv.axon_hooks, so recreate it + the ctypes hook from trn_boot)."""
    import types
    import ctypes
    import contextlib
    try:
        from antenv.axon_hooks import get_axon_ntff_profile_hook  # noqa
        return
    except ImportError:
        pass
    import antenv

    mod = types.ModuleType("antenv.axon_hooks")
    _state = {"hook": None}

    def set_axon_ntff_profile_hook(h):
        _state["hook"] = h

    def get_axon_ntff_profile_hook():
        return _state["hook"]

    mod.set_axon_ntff_profile_hook = set_axon_ntff_profile_hook
    mod.get_axon_ntff_profile_hook = get_axon_ntff_profile_hook
    sys.modules["antenv.axon_hooks"] = mod
    antenv.axon_hooks = mod

    so_path = "/opt/axon/libaxon_pjrt.so"
    lib = ctypes.CDLL(so_path)
    if not hasattr(lib, "axon_start_nrt_profile"):
        return
    lib.axon_start_nrt_profile.argtypes = [ctypes.POINTER(ctypes.c_int64), ctypes.c_size_t]
    lib.axon_start_nrt_profile.restype = ctypes.c_int64
    lib.axon_stop_nrt_profile.argtypes = [ctypes.c_char_p]
    lib.axon_stop_nrt_profile.restype = ctypes.c_int64

    @contextlib.contextmanager
    def _hook(output_dir, device_ids):
        import jax
        jax.devices()
        if device_ids:
            ids = (ctypes.c_int64 * len(device_ids))(*device_ids)
            rc = lib.axon_start_nrt_profile(ids, len(device_ids))
        else:
            rc = lib.axon_start_nrt_profile(None, 0)
        if rc != 0:
            raise RuntimeError(f"axon_start_nrt_profile rc={rc}")
        try:
            yield
        finally:
            n = lib.axon_stop_nrt_profile(str(output_dir).encode())
            print(f"ntff profile: {n} file(s) written to {output_dir}")

    set_axon_ntff_profile_hook(_hook)


def _st_rows4(s, e, T, base):
    # 4 windows of size 2^k (largest even k with 2^k <= L) covering [s, e]
    s = s.astype(np.int64)
    e = e.astype(np.int64)
    L = e - s + 1
    k = np.floor(np.log2(L.astype(np.float64))).astype(np.int64)
    k = (k // 2) * 2
    w = np.left_shift(1, k)
    rows = []
    for i in range(4):
        p = s + (i * (L - w)) // 3
        rows.append((base + (k // 2) * T + p).astype(np.int32))
    return rows


def kernel(frame_level_feat, loc_feat, conf_feat, segments, frame_segments, params):
    global LAST_EXEC_NS, LAST_RESULTS
    from ml_dtypes import bfloat16
    from concourse import bass_utils

    flf = np.asarray(frame_level_feat, np.float32)
    lf = np.asarray(loc_feat, np.float32)
    cf = np.asarray(conf_feat, np.float32)
    seg = np.asarray(segments, np.int64)
    fseg = np.asarray(frame_segments, np.int64)

    def g(d, k):
        return np.asarray(d[k], np.float32)

    in_maps = []
    for core in range(8):
        br = "loc" if core < 4 else "conf"
        p = params[br]
        s = core % 4
        bp, q = core // 4, core % 4
        sl = slice(128 * s, 128 * s + 128)
        r1 = slice(128 * s, 128 * s + 128)
        r2 = slice(512 + 128 * s, 512 + 128 * s + 128)

        cols = np.concatenate([
            np.arange(128 * s, 128 * s + 128),            # roi block
            512 + np.arange(128 * s, 128 * s + 128),      # lr start-half
            1024 + np.arange(128 * s, 128 * s + 128),     # lr end-half
            1536 + np.arange(128 * s, 128 * s + 128),     # cur block
        ])
        whead = np.zeros((512, 24), np.float32)
        wcen = np.zeros((512, 3), np.float32)
        if br == "loc":
            whead[:, 0:2] = g(params, "loc_head_w").T
            wcen[:, :] = np.asarray(params["center_w"], np.float32)[0]
        else:
            whead[:, 2:23] = g(params, "conf_head_w").T

        gngv = np.zeros((128, 9), np.float32)
        gnbv = np.zeros((128, 9), np.float32)
        gngv[:, 0] = g(p, "cur_g")[sl]
        gnbv[:, 0] = g(p, "cur_be")[sl]
        gngv[:, 1] = g(p, "lr_g")[r1]
        gnbv[:, 1] = g(p, "lr_be")[r1]
        gngv[:, 2] = g(p, "lr_g")[r2]
        gnbv[:, 2] = g(p, "lr_be")[r2]
        gngv[:, 3] = g(p, "roi_g")[sl]
        gnbv[:, 3] = g(p, "roi_be")[sl]
        for m in range(4):
            gngv[:, 4 + m] = g(p, "prop_g")[128 * m:128 * m + 128]
            gnbv[:, 4 + m] = g(p, "prop_be")[128 * m:128 * m + 128]

        gx = np.zeros((128, 20), np.int32)
        side = 0 if q < 2 else 1
        rows = _st_rows4(fseg[bp, :, 2 * side], fseg[bp, :, 2 * side + 1], TF, 0)
        for j in range(4):
            gx[:, j] = rows[j]
        for b in range(2):
            for half in range(2):
                sub = b * 2 + half
                rows = _st_rows4(seg[b, :, 2 * half], seg[b, :, 2 * half + 1],
                                 N, sub * NLL * N)
                for j in range(4):
                    gx[:, 4 + 4 * sub + j] = rows[j]

        f = lf if br == "loc" else cf
        in_maps.append({
            "feat": np.concatenate([f[0], f[1]], axis=1).astype(bfloat16),
            "flfs": np.ascontiguousarray(flf[bp, 128 * q:128 * q + 128, :]),
            "wcur": np.ascontiguousarray(g(p, "cur_w")[sl, :].T).astype(bfloat16),
            "wlr": np.ascontiguousarray(
                np.concatenate([g(p, "lr_w")[r1], g(p, "lr_w")[r2]], axis=0).T
            ).astype(bfloat16),
            "wroi": np.ascontiguousarray(g(p, "roi_w")[sl, :].T).astype(bfloat16),
            "wprop": np.ascontiguousarray(g(p, "prop_w")[:, cols].T).astype(bfloat16),
            "whead": whead.astype(bfloat16),
            "wcen": wcen.astype(bfloat16),
            "gng": gngv,
            "gnb": gnbv,
            "gidx": gx,
        })

    nc = _get_nc()
    if TRACE:
        _install_ntff_hook()
    res = bass_utils.run_bass_kernel_spmd(
        nc, in_maps, core_ids=list(range(8)), trace=TRACE)
    LAST_EXEC_NS = res.exec_time_ns
    LAST_RESULTS = res
    outs = res.results

    start = np.stack([np.concatenate([outs[4 * b + 0]["oA"], outs[4 * b + 1]["oA"]], axis=1)
                      for b in range(2)])
    end = np.stack([np.concatenate([outs[4 * b + 2]["oA"], outs[4 * b + 3]["oA"]], axis=1)
                    for b in range(2)])
    start_loc = np.concatenate([outs[s]["oB"] for s in range(4)], axis=2)
    end_loc = np.concatenate([outs[s]["oC"] for s in range(4)], axis=2)
    start_conf = np.concatenate([outs[4 + s]["oB"] for s in range(4)], axis=2)
    end_conf = np.concatenate([outs[4 + s]["oC"] for s in range(4)], axis=2)

    oh0 = outs[0]["oH"]
    oh4 = outs[4]["oH"]
    prop_loc = oh0[0:2].reshape(2, 2, 128).transpose(1, 2, 0) \
        + np.asarray(params["loc_head_b"], np.float32)[None, None, :]
    prop_conf = oh4[2:23].reshape(21, 2, 128).transpose(1, 2, 0) \
        + np.asarray(params["conf_head_b"], np.float32)[None, None, :]
    taps = oh0[32:35].reshape(3, 2, 128)
    cen = taps[1].copy()
    cen[:, 1:] += taps[0][:, :-1]
    cen[:, :-1] += taps[2][:, 1:]
    center = cen[:, :, None] + np.asarray(params["center_b"], np.float32)[None, None, :]

    return (start.astype(np.float32), end.astype(np.float32),
            prop_loc.astype(np.float32), prop_conf.astype(np.float32),
            center.astype(np.float32),
            start_loc.astype(np.float32), end_loc.astype(np.float32),
            start_conf.astype(np.float32), end_conf.astype(np.float32))


# revision 31
# speedup vs baseline: 1.4328x; 1.4328x over previous
"""AFSDRefineHead distributed Trainium2 kernel (8 NeuronCores).

Sharding: cores 0-3 = loc branch, 4-7 = conf branch. Within a branch group,
core s owns a 128-channel output shard of every conv. The prop conv (2048->512)
is input-sharded: partial sums are AllReduce'd within each branch group.
frame_level_feat boundary pooling is sharded (batch, channel-quarter) across
all 8 cores and exchanged with one 8-way AllGather.

Segment max pooling = bf16 sparse tables (doubling windows) staged in DRAM +
indirect-DMA gathers with host-precomputed (level, offset) row indices.
"""

import sys
import numpy as np

for _p in ("/opt/trn_rl_repo",):
    if _p not in sys.path:
        sys.path.append(_p)

P = 128
B = 2
N = 128          # proposals (= T_lvl)
TF = 512         # frame-level T
CIN = 512
NLF = 5          # stored (even) levels for TF=512 tables: k in {0,2,4,6,8}
NLL = 4          # stored (even) levels for N=128 tables: k in {0,2,4,6}
EPS = 1e-5

_NC_CACHE = {}

TRACE = False
LAST_EXEC_NS = None
LAST_RESULTS = None


def _build_nc():
    import concourse.bass as bass
    import concourse.bacc as bacc
    import concourse.tile as tile
    from concourse import mybir
    from concourse.masks import make_identity

    FP = mybir.dt.float32
    Bb = mybir.dt.bfloat16
    I32 = mybir.dt.int32
    AX = mybir.AxisListType
    ALU = mybir.AluOpType
    AF = mybir.ActivationFunctionType

    nc = bacc.Bacc(None, debug=False, target_bir_lowering=False)

    # ---------------- I/O ----------------
    feat = nc.dram_tensor("feat", [CIN, 2 * N], Bb, kind="ExternalInput")
    flfs = nc.dram_tensor("flfs", [P, TF], FP, kind="ExternalInput")
    wcur = nc.dram_tensor("wcur", [CIN, P], Bb, kind="ExternalInput")
    wlr = nc.dram_tensor("wlr", [CIN, 2 * P], Bb, kind="ExternalInput")
    wroi = nc.dram_tensor("wroi", [CIN, P], Bb, kind="ExternalInput")
    wprop = nc.dram_tensor("wprop", [CIN, 512], Bb, kind="ExternalInput")
    whead = nc.dram_tensor("whead", [CIN, 24], Bb, kind="ExternalInput")
    wcen = nc.dram_tensor("wcen", [CIN, 3], Bb, kind="ExternalInput")
    gng = nc.dram_tensor("gng", [P, 9], FP, kind="ExternalInput")
    gnb = nc.dram_tensor("gnb", [P, 9], FP, kind="ExternalInput")
    gidx = nc.dram_tensor("gidx", [P, 20], I32, kind="ExternalInput")

    oA = nc.dram_tensor("oA", [TF, P], FP, kind="ExternalOutput")
    oB = nc.dram_tensor("oB", [B, N, P], FP, kind="ExternalOutput")
    oC = nc.dram_tensor("oC", [B, N, P], FP, kind="ExternalOutput")
    oH = nc.dram_tensor("oH", [35, 2 * N], FP, kind="ExternalOutput")

    # ---------------- internal DRAM ----------------
    tblF = nc.dram_tensor("tblF", [NLF * TF, P], Bb)
    tblL = nc.dram_tensor("tblL", [4 * NLL * N, P], Bb)
    agin = nc.dram_tensor("agin", [P, P], Bb)
    agout = nc.dram_tensor("agout", [8 * P, P], Bb, addr_space="Shared")
    arin = nc.dram_tensor("arin", [512, 2 * N], Bb)
    arout = nc.dram_tensor("arout", [512, 2 * N], Bb)

    from contextlib import ExitStack
    with tile.TileContext(nc) as tc, ExitStack() as ctx:
        sb = ctx.enter_context(tc.tile_pool(name="sb", bufs=1))
        sb2 = ctx.enter_context(tc.tile_pool(name="sb2", bufs=2))
        pmm = ctx.enter_context(tc.tile_pool(name="pmm", bufs=2, space="PSUM"))
        ptr = ctx.enter_context(tc.tile_pool(name="ptr", bufs=2, space="PSUM"))
        pt = ctx.enter_context(tc.tile_pool(name="pt", bufs=2, space="PSUM"))

        # ---------- constants (NEFF-embedded) ----------
        from ml_dtypes import bfloat16 as np_bf16

        def const_tile(arr, dt, name):
            d = nc.inline_tensor(np.ascontiguousarray(arr), name=name)
            t = sb.tile(list(arr.shape), dt, tag=name)
            nc.sync.dma_start(out=t[:], in_=d[:, :])
            return t

        eyeF = const_tile(np.eye(P, dtype=np.float32), FP, "eyeF")
        eyeB = const_tile(np.eye(P, dtype=np.float32).astype(np_bf16), Bb, "eyeB")

        def expmat(Gn, gs, transposed):
            m = np.zeros((P, Gn), np.float32)
            for c in range(P):
                m[c, c // gs] = 1.0
            return m.T.copy() if transposed else m

        E16 = const_tile(expmat(8, 16, False), FP, "E16")
        ET16 = const_tile(expmat(8, 16, True), FP, "ET16")
        E32 = const_tile(expmat(4, 32, False), FP, "E32")
        ET32 = const_tile(expmat(4, 32, True), FP, "ET32")
        epsT = const_tile(np.full((P, 1), EPS, np.float32), FP, "epsT")

        # ---------- input loads ----------
        def load(dram, shape, dt, name, ktiles=None):
            t = sb.tile(shape, dt, tag=name)
            src = dram[:, :]
            if ktiles is not None:
                src = src.rearrange("(k p) n -> p k n", p=P)
            nc.sync.dma_start(out=t[:], in_=src)
            return t

        feat_s = load(feat, [P, 4, 2 * N], Bb, "feat_s", ktiles=4)
        flfs_s = load(flfs, [P, TF], FP, "flfs_s")
        wcur_s = load(wcur, [P, 4, P], Bb, "wcur_s", ktiles=4)
        wlr_s = load(wlr, [P, 4, 2 * P], Bb, "wlr_s", ktiles=4)
        wroi_s = load(wroi, [P, 4, P], Bb, "wroi_s", ktiles=4)
        wprop_s = load(wprop, [P, 4, 512], Bb, "wprop_s", ktiles=4)
        whead_s = load(whead, [P, 4, 24], Bb, "whead_s", ktiles=4)
        wcen_s = load(wcen, [P, 4, 3], Bb, "wcen_s", ktiles=4)
        gng_s = load(gng, [P, 9], FP, "gng_s")
        gnb_s = load(gnb, [P, 9], FP, "gnb_s")
        gidx_s = load(gidx, [P, 20], I32, "gidx_s")

        # ---------- flf transposes -> oA ----------
        oA_sb = sb.tile([P, 4, P], FP, tag="oA_sb")
        for i in range(4):
            tp = ptr.tile([P, P], FP, tag="tr")
            nc.tensor.transpose(tp[:], flfs_s[:, i * P:(i + 1) * P], eyeF[:])
            nc.scalar.copy(oA_sb[:, i, :], tp[:])
        nc.sync.dma_start(out=oA[:, :].rearrange("(i p) c -> p i c", p=P), in_=oA_sb[:])

        # ---------- flf sparse table ----------
        # All levels built channel-major (free-dim shifts only); even levels
        # PE-transposed into t-major FTf and written to DRAM for the gather.
        FTf = sb.tile([P, NLF, 4, P], Bb, tag="FTf")
        Lch = sb.tile([P, 9, TF], Bb, tag="Lch")
        nc.vector.tensor_copy(Lch[:, 0, :], flfs_s[:])
        for k in range(1, 9):
            h = 1 << (k - 1)
            nc.vector.tensor_max(Lch[:, k, 0:TF - h], Lch[:, k - 1, 0:TF - h],
                                 Lch[:, k - 1, h:TF])
            nc.vector.tensor_copy(Lch[:, k, TF - h:TF], Lch[:, k - 1, TF - h:TF])
        for li in range(NLF):
            tpb = ptr.tile([P, 4, P], Bb, tag="trb")
            for i in range(4):
                nc.tensor.transpose(tpb[:, i, :], Lch[:, 2 * li, i * P:(i + 1) * P],
                                    eyeB[:])
            if li % 2 == 0:
                nc.scalar.copy(FTf[:, li, :, :], tpb[:])
            else:
                nc.vector.tensor_copy(FTf[:, li, :, :], tpb[:])
        nc.sync.dma_start(
            out=tblF[:, :].rearrange("(k i p) c -> p k i c", p=P, i=4),
            in_=FTf[:])

        # ---------- flf gather + pool + AllGather ----------
        def gather(tbl, col, tagn):
            g = sb2.tile([P, P], Bb, tag=tagn)
            nc.gpsimd.indirect_dma_start(
                out=g[:], out_offset=None, in_=tbl[:, :],
                in_offset=bass.IndirectOffsetOnAxis(ap=gidx_s[:, col:col + 1], axis=0))
            return g

        gf = [gather(tblF, j, "gf%d" % j) for j in range(4)]
        pF = sb.tile([P, P], Bb, tag="pF")
        pFx = sb.tile([P, P], Bb, tag="pFx")
        nc.vector.tensor_max(pFx[:], gf[0][:], gf[1][:])
        nc.vector.tensor_max(pFx[:], pFx[:], gf[2][:])
        nc.vector.tensor_max(pF[:], pFx[:], gf[3][:])
        tpF = ptr.tile([P, P], Bb, tag="tr")
        nc.tensor.transpose(tpF[:], pF[:], eyeB[:])
        pFT = sb.tile([P, P], Bb, tag="pFT")
        nc.scalar.copy(pFT[:], tpF[:])
        nc.sync.dma_start(out=agin[:, :], in_=pFT[:])
        nc.gpsimd.collective_compute(
            "AllGather", ALU.bypass, replica_groups=[list(range(8))],
            ins=[agin[:, :]], outs=[agout[:, :]])
        rhsF = sb.tile([P, 4, 2 * N], Bb, tag="rhsF")
        for b in range(B):
            nc.sync.dma_start(
                out=rhsF[:, :, b * N:(b + 1) * N],
                in_=agout[b * 4 * P:(b + 1) * 4 * P, :].rearrange(
                    "(q p) n -> p q n", p=P))

        # ---------- GroupNorm helper ----------
        def gn_relu(src_ap, gcol, G, out_ap):
            # src_ap: [128, 256] f32 (psum) or bf16 (sbuf); GN per (group, batch)
            # over (128/G chans x 128 cols); ReLU; write to out_ap.
            gsz = P // G
            E, ET = (E16, ET16) if gsz == 16 else (E32, ET32)
            cnt = float(gsz * N)
            y = sb2.tile([P, 2 * N], FP, tag="gn_y")
            nc.scalar.copy(y[:], src_ap)
            ysq = sb2.tile([P, 2 * N], FP, tag="gn_ysq")
            nc.vector.tensor_tensor(ysq[:], y[:], y[:], op=ALU.mult)
            st = sb2.tile([P, 4], FP, tag="gn_st")
            nc.vector.reduce_sum(st[:, 0:2], y[:].rearrange("p (b n) -> p b n", b=2), axis=AX.X)
            nc.vector.reduce_sum(st[:, 2:4], ysq[:].rearrange("p (b n) -> p b n", b=2), axis=AX.X)
            gstat = pt.tile([G, 4], FP, tag="tiny")
            nc.tensor.matmul(gstat[:], lhsT=E[:, :], rhs=st[:], start=True, stop=True)
            gs = sb2.tile([G, 8], FP, tag="gn_gs")
            nc.scalar.mul(gs[:, 0:4], gstat[:], 1.0 / cnt)      # mu(2) | msq(2)
            nc.vector.tensor_tensor(gs[:, 4:6], gs[:, 0:2], gs[:, 0:2], op=ALU.mult)
            nc.vector.tensor_sub(gs[:, 4:6], gs[:, 2:4], gs[:, 4:6])   # var
            nc.scalar.activation(gs[:, 6:8], gs[:, 4:6], AF.Sqrt, bias=epsT[0:G, :])
            nc.vector.reciprocal(gs[:, 4:6], gs[:, 6:8])        # rstd
            nc.scalar.mul(gs[:, 6:8], gs[:, 0:2], -1.0)         # -mu
            ex = pt.tile([P, 4], FP, tag="tiny")
            nc.tensor.matmul(ex[:], lhsT=ET[:, :], rhs=gs[:, 4:8], start=True, stop=True)
            sc = sb2.tile([P, 4], FP, tag="gn_sc")
            nc.vector.tensor_scalar_mul(sc[:, 0:2], ex[:, 0:2], gng_s[:, gcol:gcol + 1])
            nc.vector.tensor_tensor(sc[:, 2:4], ex[:, 2:4], sc[:, 0:2], op=ALU.mult)
            nc.vector.tensor_scalar_add(sc[:, 2:4], sc[:, 2:4], gnb_s[:, gcol:gcol + 1])
            for b in range(B):
                nc.scalar.activation(out_ap[:, b * N:(b + 1) * N],
                                     y[:, b * N:(b + 1) * N], AF.Relu,
                                     bias=sc[:, 2 + b:3 + b], scale=sc[:, b:b + 1])

        propRhs = sb.tile([P, 4, 2 * N], Bb, tag="propRhs")
        propGN = sb.tile([P, 4, 2 * N], Bb, tag="propGN")

        # ---------- cur conv ----------
        ps = pmm.tile([P, 2 * N], FP, tag="mm")
        for k in range(4):
            nc.tensor.matmul(ps[:], lhsT=wcur_s[:, k, :], rhs=feat_s[:, k, :],
                             start=(k == 0), stop=(k == 3))
        gn_relu(ps[:], 0, 8, propRhs[:, 3, :])

        # ---------- lr conv (2 M-tiles) + feat2 outputs + lr tables ----------
        FTl = sb.tile([P, 4, NLL, P], Bb, tag="FTl")
        lrf = sb.tile([P, 2, 2 * N], FP, tag="lrf")
        for m in range(2):
            ps = pmm.tile([P, 2 * N], FP, tag="mm")
            for k in range(4):
                nc.tensor.matmul(ps[:], lhsT=wlr_s[:, k, m * P:(m + 1) * P],
                                 rhs=feat_s[:, k, :], start=(k == 0), stop=(k == 3))
            gn_relu(ps[:], 1 + m, 4, lrf[:, m, :])
        oBC = [oB, oC]
        for m in range(2):
            for b in range(B):
                tp = ptr.tile([P, P], FP, tag="tr")
                nc.tensor.transpose(tp[:], lrf[:, m, b * N:(b + 1) * N], eyeF[:])
                ob_sb = sb2.tile([P, P], FP, tag="obc")
                nc.scalar.copy(ob_sb[:], tp[:])
                nc.sync.dma_start(out=oBC[m][b, :, :], in_=ob_sb[:])
        # lr tables: all levels channel-major, even levels transposed to FTl.
        # sub-table index = b*2 + half, matching host gidx rows.
        LchL = sb.tile([P, 4, 7, N], Bb, tag="LchL")  # [ch, sub=(b,half), lvl, n]
        for b in range(B):
            for m in range(2):
                nc.vector.tensor_copy(LchL[:, b * 2 + m, 0, :],
                                      lrf[:, m, b * N:(b + 1) * N])
        for k in range(1, 7):
            h = 1 << (k - 1)
            nc.vector.tensor_max(LchL[:, :, k, 0:N - h],
                                 LchL[:, :, k - 1, 0:N - h],
                                 LchL[:, :, k - 1, h:N])
            nc.vector.tensor_copy(LchL[:, :, k, N - h:N],
                                  LchL[:, :, k - 1, N - h:N])
        for li in range(NLL):
            tpb = ptr.tile([P, 4, P], Bb, tag="trb")
            for s2 in range(4):
                nc.tensor.transpose(tpb[:, s2, :], LchL[:, s2, 2 * li, :], eyeB[:])
            if li % 2 == 0:
                nc.scalar.copy(FTl[:, :, li, :], tpb[:])
            else:
                nc.vector.tensor_copy(FTl[:, :, li, :], tpb[:])
        nc.sync.dma_start(
            out=tblL[:, :].rearrange("(s k p) c -> p s k c", p=P, k=NLL),
            in_=FTl[:])

        # ---------- lr gathers -> pooled -> prop rhs tiles 1,2 ----------
        for b in range(B):
            for half in range(2):
                sub = b * 2 + half
                gl = [gather(tblL, 4 + 4 * sub + j, "gl%d" % j) for j in range(4)]
                pl = sb2.tile([P, P], Bb, tag="pl")
                plx = sb2.tile([P, P], Bb, tag="plx")
                nc.vector.tensor_max(plx[:], gl[0][:], gl[1][:])
                nc.vector.tensor_max(plx[:], plx[:], gl[2][:])
                nc.vector.tensor_max(pl[:], plx[:], gl[3][:])
                tp = ptr.tile([P, P], Bb, tag="tr")
                nc.tensor.transpose(tp[:], pl[:], eyeB[:])
                nc.scalar.copy(propRhs[:, 1 + half, b * N:(b + 1) * N], tp[:])

        # ---------- roi conv ----------
        ps = pmm.tile([P, 2 * N], FP, tag="mm")
        for k in range(4):
            nc.tensor.matmul(ps[:], lhsT=wroi_s[:, k, :], rhs=rhsF[:, k, :],
                             start=(k == 0), stop=(k == 3))
        gn_relu(ps[:], 3, 8, propRhs[:, 0, :])

        # ---------- prop conv partials + AllReduce ----------
        # K order (1,2,3,0): the roi tile (k=0) depends on the AllGather, so
        # it accumulates last; lr/cur tiles stream in while roi finishes.
        for m in range(4):
            ps = pmm.tile([P, 2 * N], FP, tag="mm")
            for j, k in enumerate((1, 2, 3, 0)):
                nc.tensor.matmul(ps[:], lhsT=wprop_s[:, k, m * P:(m + 1) * P],
                                 rhs=propRhs[:, k, :], start=(j == 0), stop=(j == 3))
            pp = sb2.tile([P, 2 * N], Bb, tag="pp")
            nc.scalar.copy(pp[:], ps[:])
            nc.sync.dma_start(out=arin[m * P:(m + 1) * P, :], in_=pp[:])
        nc.gpsimd.collective_compute(
            "AllReduce", ALU.add, replica_groups=[[0, 1, 2, 3], [4, 5, 6, 7]],
            ins=[arin[:, :]], outs=[arout[:, :]])
        # post-AR: batched GroupNorm over all 4 prop M-tiles at once
        prop_sb = sb.tile([P, 4, 2 * N], Bb, tag="prop_sb")
        nc.sync.dma_start(out=prop_sb[:],
                          in_=arout[:, :].rearrange("(m p) n -> p m n", p=P))
        y4 = sb.tile([P, 4, 2 * N], FP, tag="y4")
        nc.scalar.copy(y4[:], prop_sb[:])
        ysq4 = sb.tile([P, 4, 2 * N], FP, tag="ysq4")
        nc.vector.tensor_tensor(ysq4[:], y4[:], y4[:], op=ALU.mult)
        st4 = sb.tile([P, 16], FP, tag="st4")
        nc.vector.reduce_sum(st4[:, 0:8],
                             y4[:].rearrange("p m (b n) -> p m b n", b=2), axis=AX.X)
        nc.vector.reduce_sum(st4[:, 8:16],
                             ysq4[:].rearrange("p m (b n) -> p m b n", b=2), axis=AX.X)
        gstat4 = pt.tile([8, 16], FP, tag="tiny")
        nc.tensor.matmul(gstat4[:], lhsT=E16[:, :], rhs=st4[:], start=True, stop=True)
        gs4 = sb.tile([8, 32], FP, tag="gs4")
        nc.scalar.mul(gs4[:, 0:16], gstat4[:], 1.0 / (16.0 * N))   # mu(8) | msq(8)
        nc.vector.tensor_tensor(gs4[:, 16:24], gs4[:, 0:8], gs4[:, 0:8], op=ALU.mult)
        nc.vector.tensor_sub(gs4[:, 16:24], gs4[:, 8:16], gs4[:, 16:24])  # var
        nc.scalar.activation(gs4[:, 24:32], gs4[:, 16:24], AF.Sqrt, bias=epsT[0:8, :])
        nc.vector.reciprocal(gs4[:, 16:24], gs4[:, 24:32])         # rstd
        nc.scalar.mul(gs4[:, 24:32], gs4[:, 0:8], -1.0)            # -mu
        ex4 = pt.tile([P, 16], FP, tag="tiny")
        nc.tensor.matmul(ex4[:], lhsT=ET16[:, :], rhs=gs4[:, 16:32],
                         start=True, stop=True)
        sc4 = sb.tile([P, 16], FP, tag="sc4")
        gamv = gng_s[:, 4:8].unsqueeze(2).broadcast_to([P, 4, 2])
        betv = gnb_s[:, 4:8].unsqueeze(2).broadcast_to([P, 4, 2])
        nc.vector.tensor_tensor(sc4[:, 0:8].rearrange("p (m b) -> p m b", m=4),
                                ex4[:, 0:8].rearrange("p (m b) -> p m b", m=4),
                                gamv, op=ALU.mult)
        nc.vector.tensor_tensor(sc4[:, 8:16], ex4[:, 8:16], sc4[:, 0:8], op=ALU.mult)
        nc.vector.tensor_tensor(sc4[:, 8:16].rearrange("p (m b) -> p m b", m=4),
                                sc4[:, 8:16].rearrange("p (m b) -> p m b", m=4),
                                betv, op=ALU.add)
        for m in range(4):
            for b in range(B):
                j = m * 2 + b
                nc.scalar.activation(propGN[:, m, b * N:(b + 1) * N],
                                     y4[:, m, b * N:(b + 1) * N], AF.Relu,
                                     bias=sc4[:, 8 + j:9 + j], scale=sc4[:, j:j + 1])

        # ---------- heads ----------
        hps = pmm.tile([24, 2 * N], FP, tag="mm")
        cps = pt.tile([3, 2 * N], FP, tag="tiny")
        for k in range(4):
            nc.tensor.matmul(hps[:], lhsT=whead_s[:, k, :], rhs=propGN[:, k, :],
                             start=(k == 0), stop=(k == 3))
            nc.tensor.matmul(cps[:], lhsT=wcen_s[:, k, :], rhs=propGN[:, k, :],
                             start=(k == 0), stop=(k == 3))
        oH_sb = sb.tile([35, 2 * N], FP, tag="oH_sb")
        nc.scalar.copy(oH_sb[0:24, :], hps[:])
        nc.scalar.copy(oH_sb[32:35, :], cps[:])
        nc.sync.dma_start(out=oH[:, :], in_=oH_sb[:])

    nc.compile()
    return nc


def _get_nc():
    if "nc" not in _NC_CACHE:
        _NC_CACHE["nc"] = _build_nc()
    return _NC_CACHE["nc"]


def _install_ntff_hook():
    """Register the axon NTFF profile hook (the agent image lacks
    antenv.axon_hooks, so recreate it + the ctypes hook from trn_boot)."""
    import types
    import ctypes
    import contextlib
    try:
        from antenv.axon_hooks import get_axon_ntff_profile_hook  # noqa
        return
    except ImportError:
        pass
    import antenv

    mod = types.ModuleType("antenv.axon_hooks")
    _state = {"hook": None}

    def set_axon_ntff_profile_hook(h):
        _state["hook"] = h

    def get_axon_ntff_profile_hook():
        return _state["hook"]

    mod.set_axon_ntff_profile_hook = set_axon_ntff_profile_hook
    mod.get_axon_ntff_profile_hook = get_axon_ntff_profile_hook
    sys.modules["antenv.axon_hooks"] = mod
    antenv.axon_hooks = mod

    so_path = "/opt/axon/libaxon_pjrt.so"
    lib = ctypes.CDLL(so_path)
    if not hasattr(lib, "axon_start_nrt_profile"):
        return
    lib.axon_start_nrt_profile.argtypes = [ctypes.POINTER(ctypes.c_int64), ctypes.c_size_t]
    lib.axon_start_nrt_profile.restype = ctypes.c_int64
    lib.axon_stop_nrt_profile.argtypes = [ctypes.c_char_p]
    lib.axon_stop_nrt_profile.restype = ctypes.c_int64

    @contextlib.contextmanager
    def _hook(output_dir, device_ids):
        import jax
        jax.devices()
        if device_ids:
            ids = (ctypes.c_int64 * len(device_ids))(*device_ids)
            rc = lib.axon_start_nrt_profile(ids, len(device_ids))
        else:
            rc = lib.axon_start_nrt_profile(None, 0)
        if rc != 0:
            raise RuntimeError(f"axon_start_nrt_profile rc={rc}")
        try:
            yield
        finally:
            n = lib.axon_stop_nrt_profile(str(output_dir).encode())
            print(f"ntff profile: {n} file(s) written to {output_dir}")

    set_axon_ntff_profile_hook(_hook)


def _st_rows4(s, e, T, base):
    # 4 windows of size 2^k (largest even k with 2^k <= L) covering [s, e]
    s = s.astype(np.int64)
    e = e.astype(np.int64)
    L = e - s + 1
    k = np.floor(np.log2(L.astype(np.float64))).astype(np.int64)
    k = (k // 2) * 2
    w = np.left_shift(1, k)
    rows = []
    for i in range(4):
        p = s + (i * (L - w)) // 3
        rows.append((base + (k // 2) * T + p).astype(np.int32))
    return rows


def kernel(frame_level_feat, loc_feat, conf_feat, segments, frame_segments, params):
    global LAST_EXEC_NS, LAST_RESULTS
    from ml_dtypes import bfloat16
    from concourse import bass_utils

    flf = np.asarray(frame_level_feat, np.float32)
    lf = np.asarray(loc_feat, np.float32)
    cf = np.asarray(conf_feat, np.float32)
    seg = np.asarray(segments, np.int64)
    fseg = np.asarray(frame_segments, np.int64)

    def g(d, k):
        return np.asarray(d[k], np.float32)

    in_maps = []
    for core in range(8):
        br = "loc" if core < 4 else "conf"
        p = params[br]
        s = core % 4
        bp, q = core // 4, core % 4
        sl = slice(128 * s, 128 * s + 128)
        r1 = slice(128 * s, 128 * s + 128)
        r2 = slice(512 + 128 * s, 512 + 128 * s + 128)

        cols = np.concatenate([
            np.arange(128 * s, 128 * s + 128),            # roi block
            512 + np.arange(128 * s, 128 * s + 128),      # lr start-half
            1024 + np.arange(128 * s, 128 * s + 128),     # lr end-half
            1536 + np.arange(128 * s, 128 * s + 128),     # cur block
        ])
        whead = np.zeros((512, 24), np.float32)
        wcen = np.zeros((512, 3), np.float32)
        if br == "loc":
            whead[:, 0:2] = g(params, "loc_head_w").T
            wcen[:, :] = np.asarray(params["center_w"], np.float32)[0]
        else:
            whead[:, 2:23] = g(params, "conf_head_w").T

        gngv = np.zeros((128, 9), np.float32)
        gnbv = np.zeros((128, 9), np.float32)
        gngv[:, 0] = g(p, "cur_g")[sl]
        gnbv[:, 0] = g(p, "cur_be")[sl]
        gngv[:, 1] = g(p, "lr_g")[r1]
        gnbv[:, 1] = g(p, "lr_be")[r1]
        gngv[:, 2] = g(p, "lr_g")[r2]
        gnbv[:, 2] = g(p, "lr_be")[r2]
        gngv[:, 3] = g(p, "roi_g")[sl]
        gnbv[:, 3] = g(p, "roi_be")[sl]
        for m in range(4):
            gngv[:, 4 + m] = g(p, "prop_g")[128 * m:128 * m + 128]
            gnbv[:, 4 + m] = g(p, "prop_be")[128 * m:128 * m + 128]

        gx = np.zeros((128, 20), np.int32)
        side = 0 if q < 2 else 1
        rows = _st_rows4(fseg[bp, :, 2 * side], fseg[bp, :, 2 * side + 1], TF, 0)
        for j in range(4):
            gx[:, j] = rows[j]
        for b in range(2):
            for half in range(2):
                sub = b * 2 + half
                rows = _st_rows4(seg[b, :, 2 * half], seg[b, :, 2 * half + 1],
                                 N, sub * NLL * N)
                for j in range(4):
                    gx[:, 4 + 4 * sub + j] = rows[j]

        f = lf if br == "loc" else cf
        in_maps.append({
            "feat": np.concatenate([f[0], f[1]], axis=1).astype(bfloat16),
            "flfs": np.ascontiguousarray(flf[bp, 128 * q:128 * q + 128, :]),
            "wcur": np.ascontiguousarray(g(p, "cur_w")[sl, :].T).astype(bfloat16),
            "wlr": np.ascontiguousarray(
                np.concatenate([g(p, "lr_w")[r1], g(p, "lr_w")[r2]], axis=0).T
            ).astype(bfloat16),
            "wroi": np.ascontiguousarray(g(p, "roi_w")[sl, :].T).astype(bfloat16),
            "wprop": np.ascontiguousarray(g(p, "prop_w")[:, cols].T).astype(bfloat16),
            "whead": whead.astype(bfloat16),
            "wcen": wcen.astype(bfloat16),
            "gng": gngv,
            "gnb": gnbv,
            "gidx": gx,
        })

    nc = _get_nc()
    if TRACE:
        _install_ntff_hook()
        from concourse import bass2jax
        bass2jax.run_bass_via_pjrt(nc, in_maps, n_cores=8)  # warm-up
    res = bass_utils.run_bass_kernel_spmd(
        nc, in_maps, core_ids=list(range(8)), trace=TRACE)
    LAST_EXEC_NS = res.exec_time_ns
    LAST_RESULTS = res
    outs = res.results

    start = np.stack([np.concatenate([outs[4 * b + 0]["oA"], outs[4 * b + 1]["oA"]], axis=1)
                      for b in range(2)])
    end = np.stack([np.concatenate([outs[4 * b + 2]["oA"], outs[4 * b + 3]["oA"]], axis=1)
                    for b in range(2)])
    start_loc = np.concatenate([outs[s]["oB"] for s in range(4)], axis=2)
    end_loc = np.concatenate([outs[s]["oC"] for s in range(4)], axis=2)
    start_conf = np.concatenate([outs[4 + s]["oB"] for s in range(4)], axis=2)
    end_conf = np.concatenate([outs[4 + s]["oC"] for s in range(4)], axis=2)

    oh0 = outs[0]["oH"]
    oh4 = outs[4]["oH"]
    prop_loc = oh0[0:2].reshape(2, 2, 128).transpose(1, 2, 0) \
        + np.asarray(params["loc_head_b"], np.float32)[None, None, :]
    prop_conf = oh4[2:23].reshape(21, 2, 128).transpose(1, 2, 0) \
        + np.asarray(params["conf_head_b"], np.float32)[None, None, :]
    taps = oh0[32:35].reshape(3, 2, 128)
    cen = taps[1].copy()
    cen[:, 1:] += taps[0][:, :-1]
    cen[:, :-1] += taps[2][:, 1:]
    center = cen[:, :, None] + np.asarray(params["center_b"], np.float32)[None, None, :]

    return (start.astype(np.float32), end.astype(np.float32),
            prop_loc.astype(np.float32), prop_conf.astype(np.float32),
            center.astype(np.float32),
            start_loc.astype(np.float32), end_loc.astype(np.float32),
            start_conf.astype(np.float32), end_conf.astype(np.float32))
